# revision 18
# baseline (speedup 1.0000x reference)
"""Trainium2 kernel for nn_AdaFastFoodMergedModel.

FastFood transform: y = SCALE * Sel . H . diag(G) . Pi . H . diag(B) . x
(H = 4096-point orthonormal Walsh-Hadamard, Pi = random permutation,
Sel = row subset of size 1228).

Strategy: everything right of `x` is a fixed linear operator built from the
small inputs (B, G, Pi, row_idx), so fold it on the host into one dense
matrix W [4096, 1228] (bf16) and run y = x @ W on the TensorEngine.

The host also pre-casts x to bf16 and pre-arranges it into the transposed
SBUF tile layout xt[p, rt, kc, j] = x[rt*128+j, kc*128+p], so the device
does no cast and no xbar transpose at all. (On-device DMA transposes act
as serialization barriers against all concurrent DMA -- the deadlock guard
-- and were the dominant stall in every profile.)

Device work per core (rows sharded 8192/8 = 1024 rows):
  - xt row-tile loads [128, 32, 128] bf16 (1 MiB each) on the scalar HWDGE
    ring; W group tiles [128, 4, 1228] + y stores on the gpsimd SWDGE ring
    (separate queues, separate sem-lane rotations)
  - kc-outer matmul loop: 3 PSUM accumulators (512/512/204 sel columns)
    stay open across kc, lhsT loaded once per kc
  - evacuate psum -> SBUF split across DVE/ACT, DMA out f32 on gpsimd
No cross-core communication (data parallel over rows).
"""

import math
import sys

import numpy as np

sys.path.insert(0, "/opt/trn_rl_repo")

import ml_dtypes

ROWS, D = 8192, 4096
M = 1228
SCALE = math.sqrt(D / M)
N_CORES = 8
SHARD = ROWS // N_CORES  # 1024
P = 128
KC = D // P  # 32 contraction chunks
RT = SHARD // P  # 8 row tiles per core
SEL_CHUNKS = [(0, 512), (512, 512), (1024, 204)]  # 1228 = 512+512+204
WGROUPS = 8
WG = KC // WGROUPS  # 4 kc per W group

# set by test harness to collect a profile
TRACE = False
LAST = {}

_CACHE = {}


def _fwht_cols(a: np.ndarray) -> np.ndarray:
    """Orthonormal FWHT along axis 0 (Sylvester/natural order)."""
    n = a.shape[0]
    x = a.copy()
    h = 1
    while h < n:
        x = x.reshape(n // (2 * h), 2, h, -1)
        lo = x[:, 0]
        hi = x[:, 1]
        x = np.stack((lo + hi, lo - hi), axis=1).reshape(n, -1)
        h *= 2
    return x * (1.0 / math.sqrt(n))


def _build_w(B, G, Pi, row_idx) -> np.ndarray:
    """W such that y = x @ W  (float32)."""
    S = np.zeros((D, M), dtype=np.float64)
    S[row_idx, np.arange(M)] = 1.0  # Sel^T
    A = _fwht_cols(S)  # H .
    A = A * G[:, None].astype(np.float64)  # diag(G) .
    A2 = np.empty_like(A)
    A2[Pi] = A  # Pi^T .
    A2 = _fwht_cols(A2)  # H .
    W = SCALE * (B[:, None].astype(np.float64) * A2)  # diag(B) .
    return W.astype(np.float32)


def _install_ntff_shim():
    """The image's antenv lacks axon_hooks; provide it so
    run_bass_kernel_spmd(trace=True) can collect an NTFF profile."""
    import types

    try:
        import antenv.axon_hooks  # noqa: F401

        return
    except ImportError:
        pass
    try:
        from trn_agent_boot.trn_boot import _ntff_profile_via_ctypes

        hook = _ntff_profile_via_ctypes("/opt/axon/libaxon_pjrt.so")
    except Exception:
        hook = None
    mod = types.ModuleType("antenv.axon_hooks")
    mod.get_axon_ntff_profile_hook = lambda: hook
    mod.set_axon_ntff_profile_hook = lambda h: None
    sys.modules["antenv.axon_hooks"] = mod


def _build_bass():
    import concourse.bass as bass
    import concourse.bacc as bacc
    import concourse.mybir as mybir
    from concourse import tile

    f32 = mybir.dt.float32
    bf16 = mybir.dt.bfloat16

    nc = bacc.Bacc("TRN2", target_bir_lowering=False, debug=False)
    # xt[p, rt, kc, j] = x[rt*128+j, kc*128+p] in bf16 (host pre-arranged)
    xt_in = nc.declare_dram_parameter("xt", [P, RT, KC, P], bf16, isOutput=False)
    # W pre-arranged on host to the SBUF layout [p, kc, m]
    w_in = nc.declare_dram_parameter("w", [P, KC, M], bf16, isOutput=False)
    out = nc.declare_dram_parameter("out", [SHARD, M], f32, isOutput=True)

    with tile.TileContext(nc) as tc:
        with (
            tc.tile_pool(name="const", bufs=1) as const_pool,
            tc.tile_pool(name="xtp", bufs=3) as xt_pool,
            tc.tile_pool(name="y", bufs=2) as y_pool,
            tc.tile_pool(name="psy", bufs=2, space=bass.MemorySpace.PSUM) as psy_pool,
        ):
            w_tiles = [
                const_pool.tile([P, WG, M], bf16, tag=f"w{g}", name=f"w{g}")
                for g in range(WGROUPS)
            ]

            def emit_w(g):
                if g == 0:
                    # kc0 slice first so the very first matmul starts early
                    nc.gpsimd.dma_start(w_tiles[0][:, 0:1, :], w_in[:, 0:1, :])
                    nc.gpsimd.dma_start(w_tiles[0][:, 1:WG, :], w_in[:, 1:WG, :])
                    return
                nc.gpsimd.dma_start(
                    w_tiles[g][:], w_in[:, g * WG : (g + 1) * WG, :]
                )

            def emit_load(rt):
                """xt row-tile load (1 MiB, 8 KB per partition) on scalar ring.
                rt0 is split into kc-groups so the first matmul starts after
                256 KB instead of 1 MiB."""
                xt = xt_pool.tile([P, KC, P], bf16, tag="xt", name="xt")
                if rt == 0:
                    for g in range(2):
                        nc.scalar.dma_start(
                            xt[:, g * 16 : (g + 1) * 16, :],
                            xt_in[:, rt, g * 16 : (g + 1) * 16, :],
                        )
                else:
                    nc.scalar.dma_start(xt[:], xt_in[:, rt])
                return xt

            # staged issue to keep DMA sem-lane pressure low
            xts = {0: emit_load(0)}
            emit_w(0)
            emit_w(1)
            xts[1] = emit_load(1)
            emit_w(2)
            xts[2] = emit_load(2)
            emit_w(3)

            for rt in range(RT):
                if rt == 0:
                    emit_w(4)
                if 0 < rt and rt + 2 < RT:
                    xts[rt + 2] = emit_load(rt + 2)

                xt = xts[rt]
                if rt + 1 < RT:
                    psys = []
                    for i, (off, sz) in enumerate(SEL_CHUNKS):
                        psys.append(
                            psy_pool.tile([P, sz], f32, tag=f"psy{i}", name=f"psy{i}")
                        )
                    for kc in range(KC):
                        # stagger remaining W-group issue within rt0 so each
                        # is emitted (program order) before its first use
                        if rt == 0 and kc in (4, 8, 12):
                            emit_w(4 + kc // 4)
                        lhsT = xt[:, kc, :]
                        wsl = w_tiles[kc // WG]
                        for i, (off, sz) in enumerate(SEL_CHUNKS):
                            nc.tensor.matmul(
                                psys[i][:],
                                lhsT,
                                wsl[:, kc % WG, off : off + sz],
                                start=(kc == 0),
                                stop=(kc == KC - 1),
                            )
                    y_sb = y_pool.tile([P, M], f32)
                    nc.vector.tensor_copy(y_sb[:, 0:512], psys[0][:])
                    nc.scalar.copy(y_sb[:, 512:1024], psys[1][:])
                    nc.vector.tensor_copy(y_sb[:, 1024:1228], psys[2][:])
                    nc.gpsimd.dma_start(out[rt * P : (rt + 1) * P, :], y_sb[:])
                else:
                    # last row tile: sel-outer so each chunk's evac + store
                    # overlaps the next chunk's matmuls (shorter tail)
                    y_sb = y_pool.tile([P, M], f32)
                    for i, (off, sz) in enumerate(SEL_CHUNKS):
                        psy = psy_pool.tile([P, sz], f32, tag=f"psy{i}", name=f"psy{i}")
                        for kc in range(KC):
                            nc.tensor.matmul(
                                psy[:],
                                xt[:, kc, :],
                                w_tiles[kc // WG][:, kc % WG, off : off + sz],
                                start=(kc == 0),
                                stop=(kc == KC - 1),
                            )
                        if i == 1:
                            nc.scalar.copy(y_sb[:, off : off + sz], psy[:])
                        else:
                            nc.vector.tensor_copy(y_sb[:, off : off + sz], psy[:])
                        nc.gpsimd.dma_start(
                            out[rt * P : (rt + 1) * P, off : off + sz],
                            y_sb[:, off : off + sz],
                        )

    nc.compile()
    return nc


def kernel(x, B, G, Pi, row_idx):
    x = np.ascontiguousarray(np.asarray(x, dtype=np.float32))
    B = np.asarray(B, dtype=np.float32)
    G = np.asarray(G, dtype=np.float32)
    Pi = np.asarray(Pi, dtype=np.int32)
    row_idx = np.asarray(row_idx, dtype=np.int32)

    W = _build_w(B, G, Pi, row_idx).astype(ml_dtypes.bfloat16)
    # rearrange to SBUF layout [p, kc, m]: W[kc*128+p, m] -> Wp[p, kc, m]
    Wp = np.ascontiguousarray(W.reshape(KC, P, M).transpose(1, 0, 2))

    # host-side cast + transpose of x into the lhsT tile layout:
    # xt[p, rt, kc, j] = x_shard[rt*128+j, kc*128+p]
    xb = x.astype(ml_dtypes.bfloat16)
    xts = [
        np.ascontiguousarray(
            xb[c * SHARD : (c + 1) * SHARD]
            .reshape(RT, P, KC, P)
            .transpose(3, 0, 2, 1)
        )
        for c in range(N_CORES)
    ]

    if "nc" not in _CACHE:
        _CACHE["nc"] = _build_bass()
    nc = _CACHE["nc"]

    if TRACE:
        _install_ntff_shim()

    from concourse.bass_utils import run_bass_kernel_spmd

    in_maps = [{"xt": xts[i], "w": Wp} for i in range(N_CORES)]

    res = run_bass_kernel_spmd(
        nc, in_maps, core_ids=list(range(N_CORES)), trace=TRACE
    )
    LAST["exec_time_ns"] = getattr(res, "exec_time_ns", None)
    LAST["results"] = res

    outs = [np.asarray(res.results[i]["out"]) for i in range(N_CORES)]
    return np.concatenate(outs, axis=0).astype(np.float32)


if __name__ == "__main__":
    rng = np.random.default_rng(0)
    x = rng.standard_normal((ROWS, D), dtype=np.float32)
    B = (rng.integers(0, 2, D) * 2 - 1).astype(np.float32)
    G = rng.standard_normal(D, dtype=np.float32)
    Pi = rng.permutation(D).astype(np.int32)
    row_idx = rng.permutation(D)[:M].astype(np.int32)
    y = kernel(x=x, B=B, G=G, Pi=Pi, row_idx=row_idx)
    print("out", y.shape, y.dtype)


# revision 19
# speedup vs baseline: 1.1759x; 1.1759x over previous
"""Trainium2 kernel for nn_AdaFastFoodMergedModel.

FastFood transform: y = SCALE * Sel . H . diag(G) . Pi . H . diag(B) . x
(H = 4096-point orthonormal Walsh-Hadamard, Pi = random permutation,
Sel = row subset of size 1228).

Strategy: everything right of `x` is a fixed linear operator built from the
small inputs (B, G, Pi, row_idx), so fold it on the host into one dense
matrix W [4096, 1228] (bf16) and run y = x @ W on the TensorEngine.

The host also pre-casts x to bf16 and pre-arranges it into the transposed
SBUF tile layout xt[p, rt, kc, j] = x[rt*128+j, kc*128+p], so the device
does no cast and no xbar transpose at all. (On-device DMA transposes act
as serialization barriers against all concurrent DMA -- the deadlock guard
-- and were the dominant stall in every profile.)

Device work per core (rows sharded 8192/8 = 1024 rows):
  - xt row-tile loads [128, 32, 128] bf16 (1 MiB each) on the scalar HWDGE
    ring; W group tiles [128, 4, 1228] + y stores on the gpsimd SWDGE ring
    (separate queues, separate sem-lane rotations)
  - kc-outer matmul loop: 3 PSUM accumulators (512/512/204 sel columns)
    stay open across kc, lhsT loaded once per kc
  - evacuate psum -> SBUF split across DVE/ACT, DMA out f32 on gpsimd
No cross-core communication (data parallel over rows).
"""

import math
import sys

import numpy as np

sys.path.insert(0, "/opt/trn_rl_repo")

import ml_dtypes

ROWS, D = 8192, 4096
M = 1228
SCALE = math.sqrt(D / M)
N_CORES = 8
SHARD = ROWS // N_CORES  # 1024
P = 128
KC = D // P  # 32 contraction chunks
RT = SHARD // P  # 8 row tiles per core
SEL_CHUNKS = [(0, 512), (512, 512), (1024, 204)]  # 1228 = 512+512+204
WGROUPS = 8
WG = KC // WGROUPS  # 4 kc per W group

# set by test harness to collect a profile
TRACE = False
LAST = {}

_CACHE = {}


def _fwht_cols(a: np.ndarray) -> np.ndarray:
    """Orthonormal FWHT along axis 0 (Sylvester/natural order)."""
    n = a.shape[0]
    x = a.copy()
    h = 1
    while h < n:
        x = x.reshape(n // (2 * h), 2, h, -1)
        lo = x[:, 0]
        hi = x[:, 1]
        x = np.stack((lo + hi, lo - hi), axis=1).reshape(n, -1)
        h *= 2
    return x * (1.0 / math.sqrt(n))


def _build_w(B, G, Pi, row_idx) -> np.ndarray:
    """W such that y = x @ W  (float32)."""
    S = np.zeros((D, M), dtype=np.float64)
    S[row_idx, np.arange(M)] = 1.0  # Sel^T
    A = _fwht_cols(S)  # H .
    A = A * G[:, None].astype(np.float64)  # diag(G) .
    A2 = np.empty_like(A)
    A2[Pi] = A  # Pi^T .
    A2 = _fwht_cols(A2)  # H .
    W = SCALE * (B[:, None].astype(np.float64) * A2)  # diag(B) .
    return W.astype(np.float32)


def _install_ntff_shim():
    """The image's antenv lacks axon_hooks; provide it so
    run_bass_kernel_spmd(trace=True) can collect an NTFF profile."""
    import types

    try:
        import antenv.axon_hooks  # noqa: F401

        return
    except ImportError:
        pass
    try:
        from trn_agent_boot.trn_boot import _ntff_profile_via_ctypes

        hook = _ntff_profile_via_ctypes("/opt/axon/libaxon_pjrt.so")
    except Exception:
        hook = None
    mod = types.ModuleType("antenv.axon_hooks")
    mod.get_axon_ntff_profile_hook = lambda: hook
    mod.set_axon_ntff_profile_hook = lambda h: None
    sys.modules["antenv.axon_hooks"] = mod


def _build_bass():
    import concourse.bass as bass
    import concourse.bacc as bacc
    import concourse.mybir as mybir
    from concourse import tile

    f32 = mybir.dt.float32
    bf16 = mybir.dt.bfloat16

    nc = bacc.Bacc("TRN2", target_bir_lowering=False, debug=False)
    # xt[p, rt, kc, j] = x[rt*128+j, kc*128+p] in bf16 (host pre-arranged)
    xt_in = nc.declare_dram_parameter("xt", [P, RT, KC, P], bf16, isOutput=False)
    # W pre-arranged on host to the SBUF layout [p, kc, m]
    w_in = nc.declare_dram_parameter("w", [P, KC, M], bf16, isOutput=False)
    out = nc.declare_dram_parameter("out", [SHARD, M], f32, isOutput=True)

    with tile.TileContext(nc) as tc:
        with (
            tc.tile_pool(name="const", bufs=1) as const_pool,
            tc.tile_pool(name="xtp", bufs=3) as xt_pool,
            tc.tile_pool(name="y", bufs=2) as y_pool,
            tc.tile_pool(name="psy", bufs=2, space=bass.MemorySpace.PSUM) as psy_pool,
        ):
            w_tiles = [
                const_pool.tile([P, WG, M], bf16, tag=f"w{g}", name=f"w{g}")
                for g in range(WGROUPS)
            ]

            def emit_w(g):
                nc.gpsimd.dma_start(
                    w_tiles[g][:], w_in[:, g * WG : (g + 1) * WG, :]
                )

            def emit_load(rt):
                """xt row-tile load (1 MiB, 8 KB per partition) on scalar ring.
                rt0 is split into kc-groups so the first matmul starts after
                256 KB instead of 1 MiB."""
                xt = xt_pool.tile([P, KC, P], bf16, tag="xt", name="xt")
                if rt == 0:
                    for g in range(2):
                        nc.scalar.dma_start(
                            xt[:, g * 16 : (g + 1) * 16, :],
                            xt_in[:, rt, g * 16 : (g + 1) * 16, :],
                        )
                else:
                    nc.scalar.dma_start(xt[:], xt_in[:, rt])
                return xt

            # staged issue to keep DMA sem-lane pressure low
            xts = {0: emit_load(0)}
            emit_w(0)
            emit_w(1)
            xts[1] = emit_load(1)
            emit_w(2)
            xts[2] = emit_load(2)
            emit_w(3)

            for rt in range(RT):
                if rt == 0:
                    emit_w(4)
                if 0 < rt and rt + 2 < RT:
                    xts[rt + 2] = emit_load(rt + 2)

                xt = xts[rt]
                if rt + 1 < RT:
                    psys = []
                    for i, (off, sz) in enumerate(SEL_CHUNKS):
                        psys.append(
                            psy_pool.tile([P, sz], f32, tag=f"psy{i}", name=f"psy{i}")
                        )
                    for kc in range(KC):
                        # stagger remaining W-group issue within rt0 so each
                        # is emitted (program order) before its first use
                        if rt == 0 and kc in (4, 8, 12):
                            emit_w(4 + kc // 4)
                        lhsT = xt[:, kc, :]
                        wsl = w_tiles[kc // WG]
                        for i, (off, sz) in enumerate(SEL_CHUNKS):
                            nc.tensor.matmul(
                                psys[i][:],
                                lhsT,
                                wsl[:, kc % WG, off : off + sz],
                                start=(kc == 0),
                                stop=(kc == KC - 1),
                            )
                    y_sb = y_pool.tile([P, M], f32)
                    nc.vector.tensor_copy(y_sb[:, 0:512], psys[0][:])
                    nc.scalar.copy(y_sb[:, 512:1024], psys[1][:])
                    nc.vector.tensor_copy(y_sb[:, 1024:1228], psys[2][:])
                    nc.gpsimd.dma_start(out[rt * P : (rt + 1) * P, :], y_sb[:])
                else:
                    # last row tile: sel-outer so each chunk's evac + store
                    # overlaps the next chunk's matmuls (shorter tail)
                    y_sb = y_pool.tile([P, M], f32)
                    for i, (off, sz) in enumerate(SEL_CHUNKS):
                        psy = psy_pool.tile([P, sz], f32, tag=f"psy{i}", name=f"psy{i}")
                        for kc in range(KC):
                            nc.tensor.matmul(
                                psy[:],
                                xt[:, kc, :],
                                w_tiles[kc // WG][:, kc % WG, off : off + sz],
                                start=(kc == 0),
                                stop=(kc == KC - 1),
                            )
                        if i == 1:
                            nc.scalar.copy(y_sb[:, off : off + sz], psy[:])
                        else:
                            nc.vector.tensor_copy(y_sb[:, off : off + sz], psy[:])
                        nc.gpsimd.dma_start(
                            out[rt * P : (rt + 1) * P, off : off + sz],
                            y_sb[:, off : off + sz],
                        )

    nc.compile()
    return nc


def kernel(x, B, G, Pi, row_idx):
    x = np.ascontiguousarray(np.asarray(x, dtype=np.float32))
    B = np.asarray(B, dtype=np.float32)
    G = np.asarray(G, dtype=np.float32)
    Pi = np.asarray(Pi, dtype=np.int32)
    row_idx = np.asarray(row_idx, dtype=np.int32)

    W = _build_w(B, G, Pi, row_idx).astype(ml_dtypes.bfloat16)
    # rearrange to SBUF layout [p, kc, m]: W[kc*128+p, m] -> Wp[p, kc, m]
    Wp = np.ascontiguousarray(W.reshape(KC, P, M).transpose(1, 0, 2))

    # host-side cast + transpose of x into the lhsT tile layout:
    # xt[p, rt, kc, j] = x_shard[rt*128+j, kc*128+p]
    xb = x.astype(ml_dtypes.bfloat16)
    xts = [
        np.ascontiguousarray(
            xb[c * SHARD : (c + 1) * SHARD]
            .reshape(RT, P, KC, P)
            .transpose(3, 0, 2, 1)
        )
        for c in range(N_CORES)
    ]

    if "nc" not in _CACHE:
        _CACHE["nc"] = _build_bass()
    nc = _CACHE["nc"]

    if TRACE:
        _install_ntff_shim()

    from concourse.bass_utils import run_bass_kernel_spmd

    in_maps = [{"xt": xts[i], "w": Wp} for i in range(N_CORES)]

    res = run_bass_kernel_spmd(
        nc, in_maps, core_ids=list(range(N_CORES)), trace=TRACE
    )
    LAST["exec_time_ns"] = getattr(res, "exec_time_ns", None)
    LAST["results"] = res

    outs = [np.asarray(res.results[i]["out"]) for i in range(N_CORES)]
    return np.concatenate(outs, axis=0).astype(np.float32)


if __name__ == "__main__":
    rng = np.random.default_rng(0)
    x = rng.standard_normal((ROWS, D), dtype=np.float32)
    B = (rng.integers(0, 2, D) * 2 - 1).astype(np.float32)
    G = rng.standard_normal(D, dtype=np.float32)
    Pi = rng.permutation(D).astype(np.int32)
    row_idx = rng.permutation(D)[:M].astype(np.int32)
    y = kernel(x=x, B=B, G=G, Pi=Pi, row_idx=row_idx)
    print("out", y.shape, y.dtype)
